# revision 3
# baseline (speedup 1.0000x reference)
"""Trainium2 Bass kernel for nn_ByteBitwiseFFN.

Reference semantics (per token, D=128 features):
  a = argmax(x[4:20]) + 16*argmax(x[20:36])
  b = argmax(x[36:52]) + 16*argmax(x[52:68])
  res = AND/OR/XOR LUT[a,b] picked by flags x[1]>0.5 / x[2]>0.5 / x[3]>0.5
        (priority AND, OR, XOR; XOR value also used when no flag set)
  active = (x[0]>=0.5) & any-flag; w = active ? 2 : 0
  out = x; out[68 + (res&15)] += w; out[84 + (res>>4)] += w

Key identities:
* Bitwise ops factor over nibbles, so the 256x256 LUTs are never needed:
  res&15 = op(a_lo, b_lo), res>>4 = op(a_hi, b_hi), and for 4-bit operands
  op(u, v) = alpha*(u+v) + beta*(u AND v) with (alpha, beta) =
  (0,1) AND / (1,-1) OR / (1,-2) XOR.  The AND is one int16 bitwise_and.
* First-occurrence argmax without any compare instruction:
  min over j of ((max - x_j) + (j-16)*2^-100) is (argmax-16)*2^-100 --
  the subtract term is 0 only at max positions and otherwise dwarfs the
  index encoding (value gaps in the data are > 1e-6 >> 16*2^-100).

Sharding: pure data parallel over tokens; each of the 8 cores gets
131072/8 = 16384 tokens as its own ExternalInput.  Engine split: DVE
keeps the two reductions + compares + casts, GpSimd takes the big
subtract pass + arithmetic algebra + f32 accumulates, the Scalar (ACT)
engine takes the affine rescales.  DMA: contiguous 2MB chunk transfers.
"""

import sys

if "/opt/trn_rl_repo" not in sys.path:
    sys.path.insert(0, "/opt/trn_rl_repo")

import numpy as np

B, S, D = 16, 8192, 128
N_CORES = 8
TOK = B * S                      # 131072 tokens
TOK_PER_CORE = TOK // N_CORES    # 16384
P = 128                          # SBUF partitions

OUT_LO, OUT_HI = 68, 84
EPS = 2.0 ** -100                # index encoding scale for the argmax trick


def build_program(tok_per_core=TOK_PER_CORE, t_per_chunk=32):
    """Build + compile the single-core SPMD Bass program.

    Layout: the core's [tok_per_core, 128] slab is processed in chunks of
    128*T tokens.  Each chunk is one contiguous DRAM block DMA'd to an
    SBUF tile [128, T*128] (partition p holds T consecutive tokens).
    """
    import concourse.bass as bass  # noqa: F401
    from concourse import bacc, mybir, tile

    f32 = mybir.dt.float32
    bf16 = mybir.dt.bfloat16
    i16 = mybir.dt.int16
    i32 = mybir.dt.int32
    Op = mybir.AluOpType
    AF = mybir.ActivationFunctionType
    X = mybir.AxisListType.X

    T = t_per_chunk
    chunk_tok = P * T
    assert tok_per_core % chunk_tok == 0
    n_chunks = tok_per_core // chunk_tok

    nc = bacc.Bacc(
        "TRN2",
        target_bir_lowering=False,
        debug=False,
        enable_asserts=True,
        num_devices=N_CORES,
    )
    x_dram = nc.dram_tensor("x", [tok_per_core, D], f32, kind="ExternalInput").ap()
    y_dram = nc.dram_tensor("y", [tok_per_core, D], f32, kind="ExternalOutput").ap()

    with tile.TileContext(nc) as tc:
        with (
            tc.tile_pool(name="consts", bufs=1) as cpool,
            tc.tile_pool(name="xtiles", bufs=3) as xpool,
            tc.tile_pool(name="tmp", bufs=2) as tp,
        ):
            v = nc.vector
            g = nc.gpsimd
            a = nc.scalar

            # --- constants -------------------------------------------------
            idx_i = cpool.tile([P, 16], i32)
            nc.gpsimd.iota(idx_i[:], [[1, 16]], base=0, channel_multiplier=0)
            idx16 = cpool.tile([P, 16], bf16)
            v.tensor_copy(idx16[:], idx_i[:])
            # (idx - 16) * 2^-100, exact in bf16
            idxe = cpool.tile([P, 16], bf16)
            v.tensor_scalar(idxe[:], idx16[:], -16.0, EPS, Op.add, Op.mult)

            idxe_b = idxe.unsqueeze(1).unsqueeze(1).broadcast_to([P, T, 4, 16])
            idx16_b = idx16.unsqueeze(1).broadcast_to([P, T, 16])

            for i in range(n_chunks):
                xt = xpool.tile([P, T * D], f32, name="xt")
                src = x_dram[i * chunk_tok : (i + 1) * chunk_tok, :].rearrange(
                    "(p t) f -> p (t f)", p=P
                )
                nc.sync.dma_start(xt[:], src)

                x3 = xt.rearrange("p (t f) -> p t f", f=D)
                # the four 16-wide argmax fields: g=0 a_lo, 1 a_hi, 2 b_lo, 3 b_hi
                nib = x3[:, :, 4:68].rearrange("p t (g n) -> p t g n", n=16)

                # --- first-occurrence argmax of each field -----------------
                rmax = tp.tile([P, T * 4], f32, name="rmax")
                rmax3 = rmax.rearrange("p (t g) -> p t g", g=4)
                v.tensor_reduce(rmax3, nib, axis=X, op=Op.max)

                dsub = tp.tile([P, T * 64], bf16, name="dsub")
                dsub4 = dsub.rearrange("p (t g n) -> p t g n", g=4, n=16)
                g.tensor_tensor(
                    dsub4,
                    rmax3.unsqueeze(3).broadcast_to([P, T, 4, 16]),
                    nib,
                    Op.subtract,
                )

                cand = tp.tile([P, T * 64], bf16, name="cand")
                cand4 = cand.rearrange("p (t g n) -> p t g n", g=4, n=16)
                v.tensor_tensor(cand4, dsub4, idxe_b, Op.add)

                # am = (argmax - 16) * 2^-100
                am = tp.tile([P, T * 4], bf16, name="am")
                am3 = am.rearrange("p (t g) -> p t g", g=4)
                v.tensor_reduce(am3, cand4, axis=X, op=Op.min)

                # nibble values 0..15: nv = am * 2^100 + 16 (ACT affine)
                nv = tp.tile([P, T * 4], bf16, name="nv")
                nv3 = nv.rearrange("p (t g) -> p t g", g=4)
                a.activation(nv3, am3, AF.Copy, bias=16.0, scale=2.0 ** 100)
                nvi = tp.tile([P, T * 4], i16, name="nvi")
                nvi3 = nvi.rearrange("p (t g) -> p t g", g=4)
                v.tensor_copy(nvi3, nv3)

                # --- AND of nibbles via int16 bitwise op -------------------
                qi = tp.tile([P, T * 2], i16, name="qi")
                qi3 = qi.rearrange("p (t f) -> p t f", f=2)
                v.tensor_tensor(qi3, nvi3[:, :, 0:2], nvi3[:, :, 2:4], Op.bitwise_and)
                qq = tp.tile([P, T * 2], bf16, name="qq")
                qq3 = qq.rearrange("p (t f) -> p t f", f=2)
                v.tensor_copy(qq3, qi3)

                ss = tp.tile([P, T * 2], bf16, name="ss")
                ss3 = ss.rearrange("p (t f) -> p t f", f=2)
                g.tensor_tensor(ss3, nv3[:, :, 0:2], nv3[:, :, 2:4], Op.add)

                # --- flags -> alpha, beta, active --------------------------
                fl = tp.tile([P, T * 4], bf16, name="fl")
                fl3 = fl.rearrange("p (t f) -> p t f", f=4)
                g.tensor_scalar(fl3, x3[:, :, 0:4], 0.5, None, Op.is_ge)
                mk = fl3[:, :, 0:1]
                ia = fl3[:, :, 1:2]
                io = fl3[:, :, 2:3]
                ix = fl3[:, :, 3:4]

                def tmp1(nm):
                    t_ = tp.tile([P, T], bf16, name=nm)
                    return t_.unsqueeze(2)  # [P, T, 1]

                alpha = tmp1("alpha")     # 1 - is_and
                a.activation(alpha, ia, AF.Copy, bias=1.0, scale=-1.0)
                s1 = tmp1("s1")           # 3 - is_or
                a.activation(s1, io, AF.Copy, bias=3.0, scale=-1.0)
                s3 = tmp1("s3")           # is_or - 2
                a.activation(s3, io, AF.Copy, bias=-2.0, scale=1.0)
                s2 = tmp1("s2")
                g.tensor_tensor(s2, ia, s1, Op.mult)
                beta = tmp1("beta")       # 1 / -1 / -2
                g.tensor_tensor(beta, s2, s3, Op.add)
                f1 = tmp1("f1")
                g.tensor_tensor(f1, ia, io, Op.add)
                f2 = tmp1("f2")
                g.tensor_tensor(f2, f1, ix, Op.add)
                f3 = tmp1("f3")           # any flag = min(1, ia+io+ix)
                g.tensor_scalar(f3, f2, 1.0, None, Op.min)
                act = tmp1("act")
                g.tensor_tensor(act, mk, f3, Op.mult)
                gof = tmp1("gof")         # 16*(1-active)
                g.tensor_scalar(gof, act, -16.0, 16.0, Op.mult, Op.add)

                # --- res = alpha*(a+b) + beta*(a&b) + 16*(1-active) --------
                def tmp2(nm):
                    t_ = tp.tile([P, T * 2], bf16, name=nm)
                    return t_.rearrange("p (t f) -> p t f", f=2)

                c1 = tmp2("c1")
                g.tensor_tensor(c1, ss3, alpha.broadcast_to([P, T, 2]), Op.mult)
                c2 = tmp2("c2")
                g.tensor_tensor(c2, qq3, beta.broadcast_to([P, T, 2]), Op.mult)
                c3 = tmp2("c3")
                g.tensor_tensor(c3, c1, gof.broadcast_to([P, T, 2]), Op.add)
                resg = tmp2("resg")
                g.tensor_tensor(resg, c3, c2, Op.add)

                # --- one-hot += 2.0 into the output fields -----------------
                for h, off in enumerate((OUT_LO, OUT_HI)):
                    eqh = tp.tile([P, T * 16], bf16, name=f"eqh{h}")
                    eqh3 = eqh.rearrange("p (t n) -> p t n", n=16)
                    v.tensor_tensor(
                        eqh3,
                        idx16_b,
                        resg[:, :, h : h + 1].broadcast_to([P, T, 16]),
                        Op.is_equal,
                    )
                    e2 = tp.tile([P, T * 16], f32, name=f"e2{h}")
                    e23 = e2.rearrange("p (t n) -> p t n", n=16)
                    a.activation(e23, eqh3, AF.Copy, bias=0.0, scale=2.0)
                    xs = x3[:, :, off : off + 16]
                    g.tensor_tensor(xs, xs, e23, Op.add)

                dst = y_dram[i * chunk_tok : (i + 1) * chunk_tok, :].rearrange(
                    "(p t) f -> p (t f)", p=P
                )
                nc.sync.dma_start(dst, xt[:])

    nc.compile()
    return nc


_compiled = None


def _get_compiled():
    global _compiled
    if _compiled is None:
        _compiled = build_program()
    return _compiled


def run_on_hw(nc, shards, trace=False, **kw):
    from concourse.bass_utils import run_bass_kernel_spmd

    return run_bass_kernel_spmd(
        nc, [{"x": s} for s in shards], list(range(N_CORES)), trace=trace, **kw
    )


def kernel(x_bd, and_table=None, or_table=None, xor_table=None):
    x = np.ascontiguousarray(np.asarray(x_bd, dtype=np.float32)).reshape(TOK, D)
    shards = [
        np.ascontiguousarray(x[c * TOK_PER_CORE : (c + 1) * TOK_PER_CORE])
        for c in range(N_CORES)
    ]
    nc = _get_compiled()
    res = run_on_hw(nc, shards)
    out = np.concatenate([res.results[c]["y"] for c in range(N_CORES)], axis=0)
    return out.reshape(B, S, D).astype(np.float32)
